# revision 1
# baseline (speedup 1.0000x reference)
"""Multi-head self-attention (B=4, T=2048, D=1024, H=16) on 8 Trainium2
NeuronCores, head-parallel (2 heads per core).

Per-core dataflow (all bf16 matmuls, fp32 PSUM accumulation):
  xT[b] (host-pretransposed [D, T] bf16) -> SBUF
  qT/kT = w_{q,k}^T @ x^T          [128=2*dk, T]   (transposed layout)
  v     = x @ w_v                  [T, 128=2*dk]   (natural layout, +ones col)
  S^T   = kT.T @ qT per (k-block, q-panel), two heads row-tiled on the PE
  causal trapezoid: strictly-upper k-blocks skipped entirely; diagonal
  superblock k-blocks restricted to their valid q-column range (N=512-128j)
  with only the [128,128] boundary block masked by a 0/1 tril multiply
  P^T   = exp(S^T / 8) on ACT over the valid columns only
  PV    = v_aug.T @ P^T -> [65, W] PSUM (row 64 = softmax denominator l)
  attn_T = PV * recip(l) (broadcast), head1 shifted to partitions 64-127
  out_partial = attn_T.T @ w_proj_rows -> HBM bf16
Host: verifies the mask is causal, pre-transposes/casts x, sums the 8
partial outputs in fp32.
"""
import numpy as np
import ml_dtypes

B, T, D, H, DK = 4, 2048, 1024, 16, 64
NCORES = 8
CD = 128          # per-core head dims (2 heads x 64)
W = 512           # q panel width
NCH = D // 128    # contraction chunks for qkv
VS = 66           # v_aug per-head stride: 64 v cols + 1 ones + 1 pad

bf16 = ml_dtypes.bfloat16
_PROG_CACHE = {}
LAST_RESULT = None


def _install_ntff_hook():
    """Register antenv.axon_hooks (NTFF profiling) if the image lacks it."""
    import contextlib
    import ctypes
    import sys
    import types

    try:
        from antenv.axon_hooks import get_axon_ntff_profile_hook  # noqa: F401
        return
    except ImportError:
        pass

    lib = ctypes.CDLL("/opt/axon/libaxon_pjrt.so")
    if not hasattr(lib, "axon_start_nrt_profile"):
        return
    lib.axon_start_nrt_profile.argtypes = [ctypes.POINTER(ctypes.c_int64), ctypes.c_size_t]
    lib.axon_start_nrt_profile.restype = ctypes.c_int64
    lib.axon_stop_nrt_profile.argtypes = [ctypes.c_char_p]
    lib.axon_stop_nrt_profile.restype = ctypes.c_int64

    @contextlib.contextmanager
    def hook(output_dir, device_ids=None):
        import jax

        jax.devices()
        if device_ids:
            ids = (ctypes.c_int64 * len(device_ids))(*device_ids)
            rc = lib.axon_start_nrt_profile(ids, len(device_ids))
        else:
            rc = lib.axon_start_nrt_profile(None, 0)
        if rc != 0:
            raise RuntimeError(f"axon_start_nrt_profile rc={rc}")
        try:
            yield
        finally:
            n = lib.axon_stop_nrt_profile(str(output_dir).encode())
            print(f"profile: {n} file(s) written to {output_dir}", file=sys.stderr)

    mod = types.ModuleType("antenv.axon_hooks")
    mod.get_axon_ntff_profile_hook = lambda: hook
    mod.set_axon_ntff_profile_hook = lambda h: None
    sys.modules["antenv.axon_hooks"] = mod
    import antenv

    antenv.axon_hooks = mod


def build_program(Bv=B, Tv=T):
    import os as _os
    KN = lambda k, d: int(_os.environ.get("MHSA_" + k, d))
    import concourse.mybir as mybir
    import concourse.tile as tile
    from concourse import bacc, library_config

    dt = mybir.dt
    f32, b16 = dt.float32, dt.bfloat16
    W = KN('W', 512)
    KPP = W // 128        # k-blocks per diagonal superblock
    TPP = W // 128        # token j-blocks per panel
    NPANEL = Tv // W
    NTOK = Tv // 128
    NKB = Tv // 128

    nc = bacc.Bacc()
    xt_d = nc.declare_dram_parameter("xt", [Bv, D, Tv], b16, isOutput=False)
    wq_d = nc.declare_dram_parameter("wq", [D, CD], b16, isOutput=False)
    wk_d = nc.declare_dram_parameter("wk", [D, CD], b16, isOutput=False)
    wv_d = nc.declare_dram_parameter("wv", [D, CD], b16, isOutput=False)
    wp_d = nc.declare_dram_parameter("wp", [CD, D], b16, isOutput=False)
    mk_d = nc.declare_dram_parameter("maskt", [128, 256], b16, isOutput=False)
    out_d = nc.declare_dram_parameter("out", [Bv, Tv, D], b16, isOutput=True)

    Exp = mybir.ActivationFunctionType.Exp

    with tile.TileContext(nc) as tc:
        with (
            tc.tile_pool(name="const", bufs=1) as constp,
            tc.tile_pool(name="xt", bufs=KN("XTBUFS", 2)) as xtp,
            tc.tile_pool(name="qk", bufs=2) as qkp,
            tc.tile_pool(name="vv", bufs=2) as vvp,
            tc.tile_pool(name="at", bufs=KN("ATBUFS", 2)) as atp,
            tc.tile_pool(name="raw", bufs=KN("RAWBUFS", 2 * NPANEL + 2)) as rawp,
            tc.tile_pool(name="pt", bufs=KN("PTBUFS", 8)) as ptp,
            tc.tile_pool(name="ell", bufs=2) as ellp,
            tc.tile_pool(name="rl", bufs=10) as rlp,
            tc.tile_pool(name="bc", bufs=KN("BCBUFS", 6)) as bcp,
            tc.tile_pool(name="stg", bufs=KN("STGBUFS", 6)) as stgp,
            tc.tile_pool(name="osb", bufs=KN("OSBBUFS", 8)) as osbp,
            tc.tile_pool(name="mm", bufs=KN("MMBUFS", 2), space="PSUM") as mmp,
            tc.tile_pool(name="qs", bufs=KN("QSBUFS", 2), space="PSUM") as qsp,
            tc.tile_pool(name="pv", bufs=KN("PVBUFS", 2), space="PSUM") as pvp,
        ):
            # --- constants: weights + causal boundary mask tile ---
            wq_sb = constp.tile([128, NCH * CD], b16, tag="wq")
            wk_sb = constp.tile([128, NCH * CD], b16, tag="wk")
            wv_sb = constp.tile([128, NCH * CD], b16, tag="wv")
            for w_d, w_sb in ((wq_d, wq_sb), (wk_d, wk_sb), (wv_d, wv_sb)):
                nc.scalar.dma_start(
                    w_sb[:].rearrange("p (c m) -> p c m", c=NCH),
                    w_d[:].rearrange("(c p) m -> p c m", p=128))
            wp_sb = constp.tile([128, D], b16, tag="wp")
            nc.scalar.dma_start(wp_sb[:], wp_d[:])
            # [128, 256]: within-block tril pattern, duplicated for both heads
            mask_sb = constp.tile([128, 256], b16, tag="mask")
            nc.scalar.dma_start(mask_sb[:], mk_d[:])

            state = {}

            def emit_qkv(b):
                # load x^T for this batch (hf-major so early panels arrive
                # first), then qT/kT [2*dk, Tv] and v_aug
                xt_sb = xtp.tile([128, NCH * Tv], b16, tag="xt")
                HT = Tv // 2
                for hf in range(2):
                    for ch in range(NCH):
                        # batch 0 startup: sync queue is empty, split the
                        # load across both queues to halve arrival time
                        eng = nc.sync if (b == 0 and KN('XT0SPLIT', 0) and ch % 2) else nc.gpsimd
                        eng.dma_start(
                            xt_sb[:, ch * Tv + hf * HT: ch * Tv + (hf + 1) * HT],
                            xt_d[b, ch * 128:(ch + 1) * 128, hf * HT:(hf + 1) * HT])
                qT = qkp.tile([128, Tv], b16, tag="qT")
                kT = qkp.tile([128, Tv], b16, tag="kT")
                v_sb = vvp.tile([128, NTOK * 2 * VS], b16, tag="v")
                vr = v_sb[:].rearrange("p (n h s) -> p n h s", h=2, s=VS)
                nc.vector.memset(vr[:, :, :, 64:65], 1.0)

                def qk_chain(w_sb, dst, p):
                    ps = mmp.tile([128, W], f32, tag="mm", name="mmqk")
                    for ch in range(NCH):
                        nc.tensor.matmul(
                            ps[:], w_sb[:, ch * CD:(ch + 1) * CD],
                            xt_sb[:, ch * Tv + p * W: ch * Tv + (p + 1) * W],
                            start=(ch == 0), stop=(ch == NCH - 1))
                    nc.vector.tensor_copy(dst[:, p * W:(p + 1) * W], ps[:])

                def v_chain(kb0):
                    vg = KN('VGRP', 4)
                    for g0 in range(kb0, kb0 + 4, vg):
                        ps = mmp.tile([128, vg * CD], f32, tag="mm", name="vps")
                        for kb in range(g0, g0 + vg):
                            for ch in range(NCH):
                                nc.tensor.matmul(
                                    ps[:, (kb - g0) * CD:(kb - g0 + 1) * CD],
                                    xt_sb[:, ch * Tv + kb * 128: ch * Tv + kb * 128 + 128],
                                    wv_sb[:, ch * CD:(ch + 1) * CD],
                                    start=(ch == 0), stop=(ch == NCH - 1))
                        nc.vector.tensor_copy(
                            vr[:, g0:g0 + vg, :, 0:64],
                            ps[:].rearrange("p (n h s) -> p n h s", h=2, s=64))

                if KN('QKVORDER', 1):
                    # panel-major: attention panel p is unblocked after the
                    # p-th triplet instead of after the whole qkv pass
                    for p in range(Tv // W):
                        qk_chain(wq_sb, qT, p)
                        qk_chain(wk_sb, kT, p)
                        v_chain(4 * p * (W // 512))
                else:
                    for w_sb, dst in ((wq_sb, qT), (wk_sb, kT)):
                        for p in range(Tv // W):
                            qk_chain(w_sb, dst, p)
                    for kb0 in range(0, NTOK, 4):
                        v_chain(kb0)
                state[b] = {"qT": qT, "kT": kT, "vr": vr}

            def emit_attention(b, panel_order):
                # PE stream software-pipelined: QK(kb+1) before PV(kb);
                # per-panel l staging so recip/scale run on DVE/gpsimd while
                # the PE continues later panels, and proj can start per panel
                st = state[b]
                qT, kT, vr = st["qT"], st["kT"], st["vr"]
                st["attnT"] = atp.tile([128, Tv], b16, tag="attnT", name="attnT")
                attnT = st["attnT"]
                # flat kb stream across panels: PV lags QK by PVLAG positions
                # globally, so the next panel's QKs cover the exp/mask latency
                # of the current panel's tail instead of the PE stalling
                seq = [(p, kb) for p in panel_order for kb in range(KPP * (p + 1))]
                PVLAG = KN('PVLAG', 3)
                pv_tiles = {}
                pend = {}

                def emit_proj_panel(p):
                    attnT_l = st["attnT"]
                    for j in range(TPP * p, TPP * p + TPP):
                        osb = osbp.tile([128, D], b16, tag="osb")
                        for n in range(D // W):
                            ps = mmp.tile([128, W], f32, tag="mm")
                            nc.tensor.matmul(
                                ps[:], attnT_l[:, j * 128:(j + 1) * 128],
                                wp_sb[:, n * W:(n + 1) * W], start=True, stop=True)
                            pc = KN('PROJCOPY', 1)
                            if pc == 0 or (pc == 1 and n == 0):
                                nc.scalar.copy(osb[:, n * W:(n + 1) * W], ps[:])
                            else:
                                nc.vector.tensor_copy(osb[:, n * W:(n + 1) * W], ps[:])
                        nc.sync.dma_start(out_d[b, j * 128:(j + 1) * 128, :], osb[:])

                def emit_scale(p):
                    pv_ps = pv_tiles.pop(p)
                    for h in range(2):
                        raw = rawp.tile([65, W], b16, tag="raw")
                        reng = nc.scalar if KN('RAWENG', 0) else None
                        if reng is not None:
                            reng.copy(raw[:], pv_ps[h][0:65, :])
                        else:
                            nc.vector.tensor_copy(raw[:], pv_ps[h][0:65, :])
                        lrow = rlp.tile([1, W], f32, tag="lrow", name=f"lr{b}{p}{h}")
                        if KN('LFROMRAW', 0):
                            # source l from the SBUF raw copy: pv psum is
                            # released after a single read, unblocking the
                            # next panel's PV accumulation sooner
                            nc.vector.tensor_copy(lrow[:], raw[64:65, :])
                        else:
                            nc.vector.tensor_copy(lrow[:], pv_ps[h][64:65, :])
                        rcp = rlp.tile([1, W], f32, tag="rcp", name=f"rc{b}{p}{h}")
                        nc.vector.reciprocal_approx_fast(rcp[:], lrow[:])
                        if KN('BCBF16', 0):
                            rcph = rlp.tile([1, W], b16, tag="rcph", name=f"rh{b}{p}{h}")
                            nc.vector.tensor_copy(rcph[:], rcp[:])
                            bc = bcp.tile([64, W], b16, tag="bch")
                            nc.gpsimd.partition_broadcast(bc[:], rcph[0:1, :], channels=64)
                        else:
                            bc = bcp.tile([64, W], f32, tag="bc")
                            nc.gpsimd.partition_broadcast(bc[:], rcp[0:1, :], channels=64)
                        if h == 0:
                            nc.vector.tensor_mul(attnT[0:64, p * W:(p + 1) * W], raw[0:64, :], bc[:])
                        else:
                            stg = stgp.tile([64, W], b16, tag="stg")
                            nc.vector.tensor_mul(stg[:], raw[0:64, :], bc[:])
                            nc.sync.dma_start(attnT[64:128, p * W:(p + 1) * W], stg[:])

                def emit_pv(p, kb):
                    pt, off = pend.pop((p, kb))
                    nkb = KPP * (p + 1)
                    if kb == 0:
                        pv_tiles[p] = [
                            pvp.tile([65, W], f32, tag="pv", name=f"pv{b}{p}{h}")
                            for h in range(2)]
                    pv_ps = pv_tiles[p]
                    for h in range(2):
                        nc.tensor.matmul(
                            pv_ps[h][0:65, off:W], vr[:, kb, h, 0:65],
                            pt[:, h * W + off: (h + 1) * W],
                            start=(kb == 0), stop=(kb == nkb - 1))
                    if kb == nkb - 1:
                        emit_scale(p)
                        ps_mode = KN('PROJSPREAD', 2)
                        if ps_mode == 1 or (ps_mode == 2 and b == Bv - 1):
                            emit_proj_panel(p)

                for i, (p, kb) in enumerate(seq):
                    # valid q-col offset within the panel (causal trapezoid)
                    j = kb - KPP * p
                    off = 128 * j if j > 0 else 0
                    qk = qsp.tile([128, 2 * W], f32, tag="qs", name="qk")
                    for h in range(2):
                        nc.tensor.matmul(
                            qk[:, h * W + off: (h + 1) * W],
                            kT[64 * h:64 * (h + 1), kb * 128:(kb + 1) * 128],
                            qT[64 * h:64 * (h + 1), p * W + off: (p + 1) * W],
                            start=True, stop=True, tile_position=(64 * h, 0))
                    if i >= PVLAG:
                        emit_pv(*seq[i - PVLAG])
                    pt = ptp.tile([128, 2 * W], b16, tag="pt")
                    if off == 0:
                        nc.scalar.activation(pt[:], qk[:], Exp, scale=0.125)
                    else:
                        qkv_view = qk[:].rearrange("p (h w) -> p h w", h=2)
                        ptv_view = pt[:].rearrange("p (h w) -> p h w", h=2)
                        nc.scalar.activation(
                            ptv_view[:, :, off:W], qkv_view[:, :, off:W],
                            Exp, scale=0.125)
                    if j >= 0:
                        # boundary block: mask the [128,128] tril frontier
                        meng = nc.gpsimd if KN('MASKGP', 0) else nc.vector
                        for h in range(2):
                            meng.tensor_mul(
                                pt[:, h * W + off: h * W + off + 128],
                                pt[:, h * W + off: h * W + off + 128],
                                mask_sb[:, h * 128:(h + 1) * 128])
                    pend[(p, kb)] = (pt, off)
                for pk in seq[-PVLAG:]:
                    emit_pv(*pk)

            def emit_proj(b, panel_order):
                ps_mode = KN('PROJSPREAD', 2)
                if ps_mode == 1 or (ps_mode == 2 and b == Bv - 1):
                    del state[b]
                    return
                attnT = state[b]["attnT"]
                for p in panel_order:
                    for j in range(TPP * p, TPP * p + TPP):
                        osb = osbp.tile([128, D], b16, tag="osb")
                        for n in range(D // W):
                            ps = mmp.tile([128, W], f32, tag="mm")
                            nc.tensor.matmul(
                                ps[:], attnT[:, j * 128:(j + 1) * 128],
                                wp_sb[:, n * W:(n + 1) * W], start=True, stop=True)
                            if (j + n) % 2 == 0:
                                nc.scalar.copy(osb[:, n * W:(n + 1) * W], ps[:])
                            else:
                                nc.vector.tensor_copy(osb[:, n * W:(n + 1) * W], ps[:])
                        nc.sync.dma_start(out_d[b, j * 128:(j + 1) * 128, :], osb[:])
                del state[b]

            # batch-level software pipeline: qkv(b+1) is emitted before
            # proj(b) so the PE never head-of-line blocks on the recip tail.
            # First batch runs panels ascending (earliest data first); the
            # last batch descending so the tail ends on the smallest panel.
            orders = {0: list(range(NPANEL)), Bv - 1: list(range(NPANEL - 1, -1, -1))}
            emit_qkv(0)
            # gpsimd ucode library (TensorTensor); loaded after batch-0 xt
            # DMA issues so they are not delayed on the gpsimd queue
            nc.gpsimd.load_library(library_config.proxy)
            sched = KN('SCHED', 0)
            if sched == 5:
                # proj(b) emitted (= prioritized) before qkv(b+1)
                for b in range(Bv):
                    order = orders.get(b, list(range(NPANEL)))
                    emit_attention(b, order)
                    emit_proj(b, order)
                    if b + 1 < Bv:
                        emit_qkv(b + 1)
            elif sched == 0:
                for b in range(Bv):
                    order = orders.get(b, list(range(NPANEL)))
                    emit_attention(b, order)
                    if b + 1 < Bv:
                        emit_qkv(b + 1)
                    emit_proj(b, order)
            elif sched == 1:
                # proj deferred one window: proj(b-1) runs under attention(b)
                for b in range(Bv):
                    order = orders.get(b, list(range(NPANEL)))
                    emit_attention(b, order)
                    if b > 0:
                        emit_proj(b - 1, orders.get(b - 1, list(range(NPANEL))))
                    if b + 1 < Bv:
                        emit_qkv(b + 1)
                emit_proj(Bv - 1, orders[Bv - 1])
            elif sched == 3:
                # interleave qkv(b+1) chain-units with proj(b) j-blocks so the
                # mm ring alternates between the two streams
                def emit_qkv_units(b):
                    units = []
                    xt_sb = xtp.tile([128, NCH * Tv], b16, tag="xt", name=f"xt{b}")
                    HT = Tv // 2
                    for hf in range(2):
                        for ch in range(NCH):
                            nc.gpsimd.dma_start(
                                xt_sb[:, ch * Tv + hf * HT: ch * Tv + (hf + 1) * HT],
                                xt_d[b, ch * 128:(ch + 1) * 128, hf * HT:(hf + 1) * HT])
                    qT = qkp.tile([128, Tv], b16, tag="qT", name=f"qT{b}")
                    kT = qkp.tile([128, Tv], b16, tag="kT", name=f"kT{b}")
                    for w_sb, dst in ((wq_sb, qT), (wk_sb, kT)):
                        for p in range(Tv // W):
                            def u(w_sb=w_sb, dst=dst, p=p):
                                ps = mmp.tile([128, W], f32, tag="mm", name="mmq")
                                for ch in range(NCH):
                                    nc.tensor.matmul(
                                        ps[:], w_sb[:, ch * CD:(ch + 1) * CD],
                                        xt_sb[:, ch * Tv + p * W: ch * Tv + (p + 1) * W],
                                        start=(ch == 0), stop=(ch == NCH - 1))
                                nc.vector.tensor_copy(dst[:, p * W:(p + 1) * W], ps[:])
                            units.append(u)
                    v_sb = vvp.tile([128, NTOK * 2 * VS], b16, tag="v", name=f"v{b}")
                    vr = v_sb[:].rearrange("p (n h s) -> p n h s", h=2, s=VS)
                    def u0(vr=vr):
                        nc.vector.memset(vr[:, :, :, 64:65], 1.0)
                    units.append(u0)
                    for kb0 in range(0, NTOK, 4):
                        def u(kb0=kb0, vr=vr):
                            ps = mmp.tile([128, 4 * CD], f32, tag="mm", name="vps")
                            for kb in range(kb0, kb0 + 4):
                                for ch in range(NCH):
                                    nc.tensor.matmul(
                                        ps[:, (kb - kb0) * CD:(kb - kb0 + 1) * CD],
                                        xt_sb[:, ch * Tv + kb * 128: ch * Tv + kb * 128 + 128],
                                        wv_sb[:, ch * CD:(ch + 1) * CD],
                                        start=(ch == 0), stop=(ch == NCH - 1))
                            nc.vector.tensor_copy(
                                vr[:, kb0:kb0 + 4, :, 0:64],
                                ps[:].rearrange("p (n h s) -> p n h s", h=2, s=64))
                        units.append(u)
                    state[b] = {"qT": qT, "kT": kT, "vr": vr}
                    return units

                def proj_units(b, order):
                    attnT = state[b]["attnT"]
                    units = []
                    for p in order:
                        for j in range(TPP * p, TPP * p + TPP):
                            def u(j=j, attnT=attnT):
                                osb = osbp.tile([128, D], b16, tag="osb", name="osbU")
                                for n in range(D // W):
                                    ps = mmp.tile([128, W], f32, tag="mm", name="mmp")
                                    nc.tensor.matmul(
                                        ps[:], attnT[:, j * 128:(j + 1) * 128],
                                        wp_sb[:, n * W:(n + 1) * W], start=True, stop=True)
                                    if n == 0:
                                        nc.scalar.copy(osb[:, n * W:(n + 1) * W], ps[:])
                                    else:
                                        nc.vector.tensor_copy(osb[:, n * W:(n + 1) * W], ps[:])
                                nc.sync.dma_start(out_d[b, j * 128:(j + 1) * 128, :], osb[:])
                            units.append(u)
                    return units

                for b in range(Bv):
                    order = orders.get(b, list(range(NPANEL)))
                    emit_attention(b, order)
                    qu = emit_qkv_units(b + 1) if b + 1 < Bv else []
                    pu = proj_units(b, order)
                    # alternate: qkv unit, proj unit, ...
                    k = 0
                    while k < max(len(qu), len(pu)):
                        if k < len(qu): qu[k]()
                        if k < len(pu): pu[k]()
                        k += 1
                    del state[b]
            elif sched == 2:
                # proj deferred; qkv before proj
                for b in range(Bv):
                    order = orders.get(b, list(range(NPANEL)))
                    emit_attention(b, order)
                    if b + 1 < Bv:
                        emit_qkv(b + 1)
                    if b > 0:
                        emit_proj(b - 1, orders.get(b - 1, list(range(NPANEL))))
                emit_proj(Bv - 1, orders[Bv - 1])

    nc.compile()
    return nc


def prep_core_inputs(x, attn_mask, w_qkv, w_proj):
    """Host-side shard prep. Returns list of 8 in_maps."""
    Bv, Tv, Dv = x.shape
    xt = np.ascontiguousarray(x.transpose(0, 2, 1)).astype(bf16)
    ql = np.arange(128)
    kl = np.arange(128)
    m1 = (ql[None, :] >= kl[:, None]).astype(bf16)
    maskt = np.concatenate([m1, m1], axis=1)  # duplicated for the 2 packed heads
    in_maps = []
    for c in range(NCORES):
        in_maps.append({
            "xt": xt,
            "wq": np.ascontiguousarray(w_qkv[:, CD * c:CD * (c + 1)]).astype(bf16),
            "wk": np.ascontiguousarray(w_qkv[:, Dv + CD * c:Dv + CD * (c + 1)]).astype(bf16),
            "wv": np.ascontiguousarray(w_qkv[:, 2 * Dv + CD * c:2 * Dv + CD * (c + 1)]).astype(bf16),
            "wp": np.ascontiguousarray(w_proj[CD * c:CD * (c + 1), :]).astype(bf16),
            "maskt": np.ascontiguousarray(maskt),
        })
    return in_maps


def check_causal(attn_mask):
    m = np.asarray(attn_mask)[0, 0]
    Tv = m.shape[0]
    tril = np.tril(np.ones((Tv, Tv), bool))
    return bool(np.all(m[tril] == 0.0)) and bool(np.all(m[~tril] <= np.float32(-1e30)))


def kernel(x, attn_mask, w_qkv, w_proj):
    import os

    from concourse.bass_utils import run_bass_kernel_spmd

    global LAST_RESULT
    x = np.asarray(x)
    attn_mask = np.asarray(attn_mask)
    w_qkv = np.asarray(w_qkv)
    w_proj = np.asarray(w_proj)
    if not check_causal(attn_mask):
        raise NotImplementedError("kernel compiled for causal attn_mask")

    key = (x.shape[0], x.shape[1])
    if key not in _PROG_CACHE:
        _PROG_CACHE[key] = build_program(Bv=x.shape[0], Tv=x.shape[1])
    nc = _PROG_CACHE[key]

    in_maps = prep_core_inputs(x, attn_mask, w_qkv, w_proj)
    kwargs = {}
    if os.environ.get("MHSA_TRACE"):
        _install_ntff_hook()
        kwargs = {"trace": True, "tmpdir": os.environ.get("MHSA_TRACE_DIR") or None}
    res = run_bass_kernel_spmd(nc, in_maps, list(range(NCORES)), **kwargs)
    LAST_RESULT = res
    out = np.zeros((x.shape[0], x.shape[1], D), np.float32)
    for c in range(NCORES):
        out += res.results[c]["out"].astype(np.float32)
    return out

